# revision 37
# baseline (speedup 1.0000x reference)
"""Trainium2 Bass kernel for nn_Attention_52536039965434 (v2).

Reference computation (B=2, SQ=SK=2048, H=1024, NH=16, HD=64):
    qkv = x @ c_attn_w + b ; per-head attention with multiplicative mask
    (post-score, pre-softmax); attn @ c_proj_w + b; gelu(cat(x, attn) @ mlp_w + b)

Sharding (8 cores): core c -> (b = c//4, g = c%4). Data parallel over batch,
tensor parallel over 4 head-groups (4 heads = 256 dims each).

v2 design vs v1:
  - all matmul operands bf16 (host converts); psum stays f32. Halves DMA,
    SBUF and collective bytes; required for the 65-wide PV matmuls.
  - PV runs q-major: out [q-tile 128, 4 heads x 65] with probs as lhsT,
    halving PV PE cycles vs the [65, q] layout. Denominator = per-head ones
    column of V. Normalize on DVE with per-partition reciprocal scalars.
  - attn tiles are PE-transposed back to feature-major for c_proj/AllGather.
  - exp on ACT in [128, 1024] tiles (2 heads per tile), bf16 out.
  - biases folded into DVE evacuations (no augmented-row matmuls).
  - gelu deferred to the tail (single ACT table switch away from Exp).
  - AllGathers split h0={qb0,qb1}, q2, q3 so c_proj/mlp2 of early segments
    fill PE gaps during later attention blocks; mlp1/Q interleaved as PE
    filler inside the exp-bound attention phase.
"""

import os

import numpy as np

import concourse.bacc as bacc
import concourse.mybir as mybir
import concourse.tile as tile
from concourse import bass_utils

# ---- problem dims (hardcoded per contest contract) ----
B = 2
S = 2048          # SQ == SK
H = 1024
NH = 16
HD = 64
NCORES = 8
TP = 4            # cores per batch (head groups)
HPC = NH // TP    # heads per core = 4
DH = HPC * HD     # per-core head width = 256
QB = 512          # q-block
P = 128
NKT = S // P      # 16 k-tiles
NQB = S // QB     # 4 q-blocks
NF = H // P       # 8 feature tiles

F32 = mybir.dt.float32
BF16 = mybir.dt.bfloat16
AF = mybir.ActivationFunctionType
ALU = mybir.AluOpType

# tail segments: one per q-block. AG1(i) fires at qb_i's end, so consumers
# sit ~2 q-blocks (50us) behind their collective - no in-order stalls.
SEGS = [(0, 1), (1, 1), (2, 1), (3, 1)]


def _build_nc(reps=1, ag_mode=None):
    if ag_mode is None:
        ag_mode = os.environ.get("KERNEL_AG", "cc")  # cc | dma (timing expt)

    nc = bacc.Bacc(
        "TRN2", target_bir_lowering=False, debug=False, num_devices=NCORES
    )

    # ---- kernel I/O (per-core contents supplied via in_maps) ----
    xatt_d = nc.dram_tensor("xatt", [H, S], BF16, kind="ExternalInput").ap()
    xatd_d = nc.dram_tensor("xatd", [H, S], BF16, kind="ExternalInput").ap()
    wq_d = nc.dram_tensor("wq", [H, DH], BF16, kind="ExternalInput").ap()
    wk_d = nc.dram_tensor("wk", [H, DH], BF16, kind="ExternalInput").ap()
    wv_d = nc.dram_tensor("wv", [H, DH], BF16, kind="ExternalInput").ap()
    qkb_d = nc.dram_tensor("qkb", [P, 4], F32, kind="ExternalInput").ap()
    vb_d = nc.dram_tensor("vb", [1, DH], BF16, kind="ExternalInput").ap()
    mask_d = nc.dram_tensor("maskrep", [P, S], BF16, kind="ExternalInput").ap()
    cpw_d = nc.dram_tensor("cprojw", [H, DH], BF16, kind="ExternalInput").ap()
    cpb_d = nc.dram_tensor("cprojb", [P, 2], F32, kind="ExternalInput").ap()
    mw_d = nc.dram_tensor("mlpw", [2 * H, DH], BF16, kind="ExternalInput").ap()
    mb_d = nc.dram_tensor("mlpb", [P, 2], F32, kind="ExternalInput").ap()
    id_d = nc.dram_tensor("ident", [P, P], BF16, kind="ExternalInput").ap()
    cpwf_d = nc.dram_tensor("cprojwf", [H, H], BF16, kind="ExternalInput").ap()
    cpbf_d = nc.dram_tensor("cprojbf", [P, NF], F32, kind="ExternalInput").ap()
    outT = nc.dram_tensor("outT", [DH, S], F32, kind="ExternalOutput").ap()

    rg = [[0, 1, 2, 3], [4, 5, 6, 7]]

    def allgather(src_ap, dst_ap):
        if ag_mode == "cc":
            nc.gpsimd.collective_compute(
                "AllGather", ALU.bypass, replica_groups=rg,
                ins=[src_ap.opt()], outs=[dst_ap.opt()],
            )
        else:
            # timing experiment: same bytes landed, no collective (wrong vals)
            for r in range(4):
                nc.gpsimd.dma_start(
                    out=dst_ap[r * DH: (r + 1) * DH, :], in_=src_ap[:])

    with tile.TileContext(nc) as tc:
      for rep in range(reps):
        with (
            tc.tile_pool(name=f"w{rep}", bufs=1) as wpool,
            tc.tile_pool(name=f"per{rep}", bufs=1) as per,
            tc.tile_pool(name=f"xch{rep}", bufs=3) as xch_pool,
            tc.tile_pool(name=f"et{rep}", bufs=4) as etp,
            tc.tile_pool(name=f"aq{rep}", bufs=2) as aqp,
            tc.tile_pool(name=f"sm{rep}", bufs=2) as smp,
            tc.tile_pool(name=f"zl{rep}", bufs=1) as zlp,
            tc.tile_pool(name=f"go{rep}", bufs=2) as gop,
            tc.tile_pool(name=f"dram{rep}", bufs=1, space="DRAM") as dram,
            tc.tile_pool(name=f"psSC{rep}", bufs=2, space="PSUM") as psSC,
            tc.tile_pool(name=f"psPV{rep}", bufs=1, space="PSUM") as psPV,
        ):
            # ---------------- weight / persistent loads ----------------
            wq_sb = wpool.tile([P, NF * DH], BF16, tag="wq")
            wk_sb = wpool.tile([P, NF * DH], BF16, tag="wk")
            wv_sb = wpool.tile([P, NF * DH], BF16, tag="wv")
            cproj_sb = wpool.tile([P, NF * DH], BF16, tag="cproj")
            mlp_sb = wpool.tile([P, 2 * NF * DH], BF16, tag="mlp")
            mask_sb = wpool.tile([P, S], BF16, tag="mask")
            ident_sb = wpool.tile([P, P], BF16, tag="ident")
            qkb_sb = wpool.tile([P, 4], F32, tag="qkb")
            cpb_sb = wpool.tile([P, 2], F32, tag="cpb")
            mb_sb = wpool.tile([P, 2], F32, tag="mb")
            vb_sb = wpool.tile([1, DH], BF16, tag="vb")
            vbb_sb = wpool.tile([P, DH], BF16, tag="vbb")
            cpwf_sb = wpool.tile([P, NF * H], BF16, tag="cpwf")
            cpbf_sb = wpool.tile([P, NF], F32, tag="cpbf")
            xatt_sb = per.tile([P, NF * S], BF16, tag="xatt")

            def load_w(w_d, w_sb):
                nc.sync.dma_start(
                    out=w_sb[:].rearrange("p (t d) -> p t d", d=DH),
                    in_=w_d[:].rearrange("(t p) d -> p t d", p=P),
                )

            def load_xatt(qb):
                cs = slice(qb * QB, (qb + 1) * QB)
                nc.sync.dma_start(
                    out=xatt_sb[:].rearrange("p (t q) -> p t q", q=S)
                    [:, :, cs],
                    in_=xatt_d[:].rearrange("(t p) q -> p t q", p=P)[:, :, cs],
                )

            def load_cold():
                # deferred until after the hot path is queued
                for qb in range(1, NQB):
                    load_xatt(qb)
                nc.sync.dma_start(
                    out=mlp_sb[:].rearrange("p (t d) -> p t d", d=DH),
                    in_=mw_d[:].rearrange("(t p) d -> p t d", p=P),
                )
                load_w(cpw_d, cproj_sb)
                nc.sync.dma_start(
                    out=cpwf_sb[:].rearrange("p (t d) -> p t d", d=H),
                    in_=cpwf_d[:].rearrange("(t p) d -> p t d", p=P),
                )
                nc.sync.dma_start(out=cpbf_sb[:], in_=cpbf_d[:])

            # persistent activations
            QT_sb = per.tile([P, 2 * S], BF16, tag="qt")    # p-half at p*S
            KT_sb = per.tile([P, 2 * S], BF16, tag="kt")
            V_sb = per.tile([P, NKT * 260], BF16, tag="v")  # kt: 4 heads x 65
            out1_sb = per.tile([P, 2 * S], BF16, tag="out1")  # mlp acc

            # ones columns of augmented V (denominator trick)
            nc.vector.memset(
                V_sb[:].rearrange("p (k h c) -> p k h c", h=HPC, c=65)
                [:, :, :, 64:65], 1.0)

            # DRAM buffers for collectives, per segment
            ag1_in, ag1_out, ag2_in, ag2_out = [], [], [], []
            for si, (q0, nqs) in enumerate(SEGS):
                L = nqs * QB
                ag1_in.append(dram.tile([DH, L], BF16, tag=f"a1i{si}",
                                        name=f"ag1_in{si}"))
                ag1_out.append(dram.tile([H, L], BF16, tag=f"a1o{si}",
                                         name=f"ag1_out{si}"))
                if si < 2:
                    ag2_in.append(dram.tile([DH, L], BF16, tag=f"a2i{si}",
                                            name=f"ag2_in{si}"))
                    ag2_out.append(dram.tile([H, L], BF16, tag=f"a2o{si}",
                                             name=f"ag2_out{si}"))
            ah_sb = [per.tile([P, NF * nqs * QB], BF16, tag=f"ah{si}",
                              name=f"ah{si}")
                     for si, (q0, nqs) in enumerate(SEGS)]
            z_sb = [per.tile([P, 2 * QB], BF16, tag=f"z{si}", name=f"z{si}")
                    for si in range(2)]

            def wsl(w_sb, t, p):
                return w_sb[:, t * DH + p * P: t * DH + (p + 1) * P]

            acc_n = [0]

            def acc_tile(w=QB):
                # fillers/K/tail GEMM accumulators live in the sc psum ring
                acc_n[0] += 1
                return psSC.tile([P, 2 * QB], F32, tag="sc",
                                 name=f"acc{acc_n[0]}")[:, 0:w]

            def emit_pv(ets, pv_tiles, kt):
                # PV for one kt step: 4 heads x 4 q-subtiles, 65-wide frees.
                # One accumulation group per psum bank: start only zeroes the
                # 2KB zero-region once (kt==0, h==0); later first-writes land
                # on pending-zero bytes, so plain accumulate is correct.
                for p in range(2):
                    et = ets[(kt, p)]
                    for h2 in range(2):
                        h = 2 * p + h2
                        for qt in range(4):
                            nc.tensor.matmul(
                                pv_tiles[qt][:, h * 65: (h + 1) * 65],
                                lhsT=et[:, h2 * QB + qt * P:
                                        h2 * QB + (qt + 1) * P],
                                rhs=V_sb[:, kt * 260 + h * 65:
                                         kt * 260 + (h + 1) * 65],
                                start=(kt == 0 and h == 0),
                                stop=(kt == NKT - 1 and h == 3),
                                skip_group_check=True)

            # ---------------- filler units (emitted lazily) -------------
            HQ = QB // 2   # 256-col filler sub-unit width

            def mlp1_unit(qb, ct, hf):
                o = qb * QB + hf * HQ
                def emit():
                    ps = acc_tile(HQ)
                    for t in range(NF):
                        nc.tensor.matmul(
                            ps[:], lhsT=wsl(mlp_sb, t, ct),
                            rhs=xatt_sb[:, t * S + o: t * S + o + HQ],
                            start=(t == 0), stop=(t == NF - 1))
                    nc.vector.tensor_scalar_add(
                        out1_sb[:, ct * S + o: ct * S + o + HQ],
                        ps[:], mb_sb[:, ct: ct + 1])
                return emit

            def q_unit(qb, p, hf):
                o = qb * QB + hf * HQ
                def emit():
                    ps = acc_tile(HQ)
                    for t in range(NF):
                        nc.tensor.matmul(
                            ps[:], lhsT=wsl(wq_sb, t, p),
                            rhs=xatt_sb[:, t * S + o: t * S + o + HQ],
                            start=(t == 0), stop=(t == NF - 1))
                    nc.vector.tensor_scalar_add(
                        QT_sb[:, p * S + o: p * S + o + HQ],
                        ps[:], qkb_sb[:, p: p + 1])
                return emit

            def cproj_unit(si, qb, ot, hf):
                q0, nqs = SEGS[si]
                qo = (qb - q0) * QB + hf * HQ
                L = nqs * QB
                def emit():
                    ps = acc_tile(HQ)
                    for t in range(NF):
                        nc.tensor.matmul(
                            ps[:], lhsT=wsl(cproj_sb, t, ot),
                            rhs=ah_sb[si][:, t * L + qo: t * L + qo + HQ],
                            start=(t == 0), stop=(t == NF - 1))
                    nc.vector.tensor_scalar_add(
                        z_sb[si][:, ot * L + qo: ot * L + qo + HQ],
                        ps[:], cpb_sb[:, ot: ot + 1])
                return emit

            def mlp2_unit(si, qb, ct, hf, zch):
                o = hf * HQ
                def emit():
                    z = zchs[qb] if zch is None else zch
                    ps = acc_tile(HQ)
                    for t in range(NF):
                        nc.tensor.matmul(
                            ps[:], lhsT=wsl(mlp_sb, NF + t, ct),
                            rhs=z[:, t * QB + o: t * QB + o + HQ],
                            start=(t == 0), stop=(t == NF - 1))
                    sl = out1_sb[:, ct * S + qb * QB + o:
                                 ct * S + qb * QB + o + HQ]
                    nc.vector.tensor_tensor(sl, ps[:], sl, ALU.add)
                return emit

            # ------------- helpers for the interleaved main phase -------
            def load_xch(kb):
                x_ch = xch_pool.tile([P, NF * QB], BF16, tag="xch",
                                     name=f"xd{kb}")
                cs = slice(kb * QB, (kb + 1) * QB)
                nc.sync.dma_start(
                    out=x_ch[:].rearrange("p (t q) -> p t q", q=QB),
                    in_=xatd_d[:].rearrange("(t p) q -> p t q", p=P)[:, :, cs],
                )
                return x_ch

            def k_group(x_ch, kb, p):
                ps = acc_tile()
                for t in range(NF):
                    nc.tensor.matmul(
                        ps[:], lhsT=wsl(wk_sb, t, p),
                        rhs=x_ch[:, t * QB: (t + 1) * QB],
                        start=(t == 0), stop=(t == NF - 1))
                # (ps + k_bias) * mask, bf16 out
                nc.vector.scalar_tensor_tensor(
                    KT_sb[:, p * S + kb * QB: p * S + (kb + 1) * QB],
                    ps[:], qkb_sb[:, 2 + p: 3 + p],
                    mask_sb[:, kb * QB: (kb + 1) * QB],
                    ALU.add, ALU.mult)

            def v_group(x_ch, kb, sub):
                kt = kb * (QB // P) + sub
                acc_n[0] += 1
                psv = psSC.tile([P, 2 * QB], F32, tag="sc",
                                name=f"psv{acc_n[0]}")[:, 0:DH]
                for t in range(NF):
                    nc.tensor.matmul(
                        psv[:],
                        lhsT=x_ch[:, t * QB + sub * P: t * QB + (sub + 1) * P],
                        rhs=wv_sb[:, t * DH: (t + 1) * DH],
                        start=(t == 0), stop=(t == NF - 1))
                nc.vector.tensor_tensor(
                    V_sb[:, kt * 260: (kt + 1) * 260]
                    .rearrange("p (h c) -> p h c", c=65)[:, :, 0:64],
                    psv.rearrange("p (h c) -> p h c", c=HD),
                    vbb_sb[:].rearrange("p (h c) -> p h c", c=HD),
                    ALU.add)

            def sc_step(qb, kt, ets):
                for p in range(2):
                    sc = psSC.tile([P, 2 * QB], F32, tag="sc",
                                   name=f"sc{qb}_{kt}_{p}")
                    for h2 in range(2):
                        nc.tensor.matmul(
                            sc[:, h2 * QB: (h2 + 1) * QB],
                            lhsT=KT_sb[64 * h2: 64 * h2 + 64,
                                       p * S + kt * P: p * S + (kt + 1) * P],
                            rhs=QT_sb[64 * h2: 64 * h2 + 64,
                                      p * S + qb * QB: p * S + (qb + 1) * QB],
                            start=True, stop=True,
                            tile_position=(64 * h2, 0))
                    et = etp.tile([P, 2 * QB], BF16, tag="et",
                                  name=f"et{qb}_{kt}_{p}")
                    nc.scalar.activation(et[:], sc[:], AF.Exp)
                    ets[(kt, p)] = et

            def seg_of(qb):
                for si, (q0, nqs) in enumerate(SEGS):
                    if q0 <= qb < q0 + nqs:
                        return si
                raise AssertionError

            def emit_ag1(si):
                allgather(ag1_in[si][:], ag1_out[si][:])
                q0, nqs = SEGS[si]
                L = nqs * QB
                nc.sync.dma_start(
                    out=ah_sb[si][:].rearrange("p (t q) -> p t q", q=L),
                    in_=ag1_out[si][:].rearrange("(t p) q -> p t q", p=P),
                )

            def emit_ag2(si):
                q0, nqs = SEGS[si]
                L = nqs * QB
                nc.sync.dma_start(
                    out=ag2_in[si][:].rearrange("(o p) q -> p o q", p=P),
                    in_=z_sb[si][:].rearrange("p (o q) -> p o q", q=L),
                )
                allgather(ag2_in[si][:], ag2_out[si][:])

            def normalize_ship(qb, si, q0, pv_tiles):
                for qt in range(4):
                    pv = pv_tiles[qt][:]
                    rec = smp.tile([P, 4], F32, tag="rec")
                    nc.vector.reciprocal(
                        rec[:].rearrange("p (h c) -> p h c", c=1),
                        pv.rearrange("p (h c) -> p h c", c=65)[:, :, 64:65])
                    aq = aqp.tile([P, DH], BF16, tag="attq")
                    for h in range(HPC):
                        nc.vector.tensor_scalar_mul(
                            aq[:, h * HD: (h + 1) * HD],
                            pv[:, h * 65: h * 65 + 64],
                            rec[:, h: h + 1])
                    # XBAR DMA transpose to feature-major, then ship to DRAM
                    at = aqp.tile([P, DH], BF16, tag="attT")
                    for f in range(2):
                        nc.sync.dma_start_transpose(
                            at[:, f * P: (f + 1) * P],
                            aq[:, f * P: (f + 1) * P])
                    qcol = (qb - q0) * QB + qt * P
                    nc.sync.dma_start(
                        out=ag1_in[si][:].rearrange("(t p) q -> p t q", p=P)
                        [:, :, qcol: qcol + P],
                        in_=at[:].rearrange("p (t q) -> p t q", q=P))

            # -------- main phase: K/V production interleaved with qb0 ------
            # hot loads first: K path, then Q path, then V
            load_w(wk_d, wk_sb)
            x_chs = {0: load_xch(0)}
            nc.sync.dma_start(out=qkb_sb[:], in_=qkb_d[:])
            nc.sync.dma_start(out=mask_sb[:], in_=mask_d[:])
            load_w(wq_d, wq_sb)
            load_xatt(0)
            x_chs[1] = load_xch(1)
            load_w(wv_d, wv_sb)
            nc.sync.dma_start(out=vb_sb[:], in_=vb_d[:])
            nc.gpsimd.partition_broadcast(vbb_sb[:], vb_sb[:], channels=P)
            nc.sync.dma_start(out=ident_sb[:], in_=id_d[:])
            nc.sync.dma_start(out=cpb_sb[:], in_=cpb_d[:])
            nc.sync.dma_start(out=mb_sb[:], in_=mb_d[:])
            k_group(x_chs[0], 0, 0)
            k_group(x_chs[0], 0, 1)
            for hf in range(2):
                q_unit(0, 0, hf)()
                q_unit(0, 1, hf)()

            pv_tiles = [
                psPV.tile([P, HPC * 65], F32, tag=f"pv{qt}", name=f"pv0_{qt}")
                for qt in range(4)]
            ets = {}
            for kb in range(NQB):
                if kb + 2 < NQB:
                    x_chs[kb + 2] = load_xch(kb + 2)
                for j in range(QB // P):
                    kt = kb * (QB // P) + j
                    sc_step(0, kt, ets)
                    if kt > 0:
                        emit_pv(ets, pv_tiles, kt - 1)
                    v_group(x_chs[kb], kb, j)
                    if kb + 1 < NQB and j in (1, 2):
                        k_group(x_chs[kb + 1], kb + 1, j - 1)
                if kb == 0:
                    load_cold()
            emit_pv(ets, pv_tiles, NKT - 1)
            for hf in range(2):
                q_unit(1, 0, hf)()
                q_unit(1, 1, hf)()
            normalize_ship(0, 0, 0, pv_tiles)

            # ---------------- qb 1..3: attention + fillers ---------------
            # filler placement: {qb: {after-kt: [unit, ...]}}
            fill = {qb: {} for qb in range(NQB)}

            def sched(qb, units, kt0=1):
                for i, u in enumerate(units):
                    fill[qb].setdefault(kt0 + i, []).append(u)

            zchs = {}

            def zch_load(si, qb):
                def emit():
                    zch = zlp.tile([P, NF * QB], BF16, tag="zch",
                                   name=f"zch{qb}")
                    nc.sync.dma_start(
                        out=zch[:].rearrange("p (t q) -> p t q", q=QB),
                        in_=ag2_out[si][:].rearrange("(t p) q -> p t q", p=P))
                    zchs[qb] = zch
                return emit

            def zfull_group(si, ot8, dst):
                # full c_proj for one 128-row output tile of a 512-q segment
                def emit():
                    ps = acc_tile()
                    for t in range(NF):
                        nc.tensor.matmul(
                            ps[:],
                            lhsT=cpwf_sb[:, t * H + ot8 * P:
                                         t * H + (ot8 + 1) * P],
                            rhs=ah_sb[si][:, t * QB: (t + 1) * QB],
                            start=(t == 0), stop=(t == NF - 1))
                    nc.vector.tensor_scalar_add(
                        dst[:, ot8 * QB: (ot8 + 1) * QB],
                        ps[:], cpbf_sb[:, ot8: ot8 + 1])
                return emit

            z23f = zlp.tile([P, NF * QB], BF16, tag="z23f")

            sched(1, [mlp1_unit(0, 0, 0), mlp1_unit(0, 0, 1),
                      mlp1_unit(0, 1, 0), mlp1_unit(0, 1, 1),
                      q_unit(2, 0, 0), q_unit(2, 0, 1),
                      q_unit(2, 1, 0), q_unit(2, 1, 1),
                      mlp1_unit(1, 0, 0), mlp1_unit(1, 0, 1),
                      mlp1_unit(1, 1, 0), mlp1_unit(1, 1, 1),
                      mlp1_unit(2, 0, 0), mlp1_unit(2, 0, 1)])
            sched(2, [q_unit(3, 0, 0), q_unit(3, 0, 1),
                      q_unit(3, 1, 0), q_unit(3, 1, 1),
                      cproj_unit(0, 0, 0, 0), cproj_unit(0, 0, 0, 1),
                      cproj_unit(0, 0, 1, 0), cproj_unit(0, 0, 1, 1)])
            fill[2].setdefault(8, []).append(lambda: emit_ag2(0))
            sched(2, [mlp1_unit(2, 1, 0), mlp1_unit(2, 1, 1),
                      mlp1_unit(3, 0, 0), mlp1_unit(3, 0, 1),
                      mlp1_unit(3, 1, 0), mlp1_unit(3, 1, 1)], kt0=9)
            fill[3].setdefault(1, []).append(zch_load(0, 0))
            sched(3, [cproj_unit(1, 1, 0, 0), cproj_unit(1, 1, 0, 1),
                      cproj_unit(1, 1, 1, 0), cproj_unit(1, 1, 1, 1)])
            fill[3].setdefault(4, []).append(lambda: emit_ag2(1))
            sched(3, [mlp2_unit(0, 0, 0, 0, None), mlp2_unit(0, 0, 0, 1, None),
                      mlp2_unit(0, 0, 1, 0, None), mlp2_unit(0, 0, 1, 1, None)],
                  kt0=5)
            # q2 z via local full c_proj (ah2 ready early in qb3)
            sched(3, [zfull_group(2, i, z23f) for i in range(6)], kt0=9)

            for qb in range(1, NQB):
                si = seg_of(qb)
                q0, nqs = SEGS[si]
                pv_tiles = [
                    psPV.tile([P, HPC * 65], F32, tag=f"pv{qt}",
                              name=f"pv{qb}_{qt}")
                    for qt in range(4)]
                ets = {}
                for kt in range(NKT):
                    sc_step(qb, kt, ets)
                    if kt > 0:
                        emit_pv(ets, pv_tiles, kt - 1)
                        for e in fill[qb].get(kt - 1, []):
                            e()
                emit_pv(ets, pv_tiles, NKT - 1)
                for e in fill[qb].get(NKT - 1, []):
                    e()
                normalize_ship(qb, si, q0, pv_tiles)
                if qb == q0 + nqs - 1:
                    emit_ag1(si)

            # ---------------- tail ----------------
            for i in range(6, NF):
                zfull_group(2, i, z23f)()
            zch_load(1, 1)()
            for ct in range(2):
                for hf in range(2):
                    mlp2_unit(1, 1, ct, hf, None)()    # qb1
            for ct in range(2):
                for hf in range(2):
                    mlp2_unit(2, 2, ct, hf, z23f)()    # q2
            gelu_exact = os.environ.get("KERNEL_GELU", "builtin") == "exact"

            def gelu_out(qb, ct):
                go = gop.tile([P, QB], F32, tag="gout")
                gin = out1_sb[:, ct * S + qb * QB: ct * S + (qb + 1) * QB]
                if not gelu_exact:
                    nc.scalar.activation(go[:], gin, AF.Gelu_apprx_tanh)
                else:
                    # exact GPT-2 tanh gelu from primitives (CoreSim path)
                    u = gop.tile([P, QB], F32, tag="gu")
                    nc.vector.tensor_mul(u[:], gin, gin)             # x^2
                    nc.vector.tensor_mul(u[:], u[:], gin)            # x^3
                    nc.vector.scalar_tensor_tensor(
                        u[:], u[:], 0.044715, gin, ALU.mult, ALU.add)
                    nc.scalar.activation(
                        go[:], u[:], AF.Tanh, scale=0.7978845608028654)
                    nc.vector.scalar_tensor_tensor(
                        go[:], go[:], 1.0, gin, ALU.add, ALU.mult)   # (1+t)*x
                    nc.vector.tensor_scalar_mul(go[:], go[:], 0.5)
                nc.sync.dma_start(
                    out=outT[ct * P: (ct + 1) * P, qb * QB: (qb + 1) * QB],
                    in_=go[:])

            for qb in range(3):
                for ct in range(2):
                    gelu_out(qb, ct)
            # q3 z via local full c_proj (reuses z23f after mlp2_q2 reads)
            for i in range(NF):
                zfull_group(3, i, z23f)()
            for ct in range(2):
                for hf in range(2):
                    mlp2_unit(3, 3, ct, hf, z23f)()
            for ct in range(2):
                gelu_out(3, ct)

    nc.compile()
    return nc


_NC_CACHE = {}
LAST_RESULTS = None


def _get_nc():
    if 1 not in _NC_CACHE:
        _NC_CACHE[1] = _build_nc()
    return _NC_CACHE[1]


def _get_nc_reps(reps):
    key = ("reps", reps)
    if key not in _NC_CACHE:
        _NC_CACHE[key] = _build_nc(reps=reps)
    return _NC_CACHE[key]


def kernel(**inputs):
    global LAST_RESULTS
    nc = _get_nc()
    in_maps = make_in_maps(inputs)

    trace = bool(int(os.environ.get("KERNEL_TRACE", "0")))
    res = bass_utils.run_bass_kernel_spmd(
        nc, in_maps, core_ids=list(range(NCORES)), trace=trace
    )
    LAST_RESULTS = res

    out = np.empty((B, S, H), np.float32)
    for c in range(NCORES):
        b, g = c // TP, c % TP
        out[b, :, g * DH: (g + 1) * DH] = res.results[c]["outT"].T
    return out


def make_in_maps(inputs):
    bf = mybir.dt.np(BF16)
    xq = np.asarray(inputs["attender_seq"], np.float32)
    xk = np.asarray(inputs["attendee_seq"], np.float32)
    mask = np.asarray(inputs["attendee_mask"]).astype(np.float32)
    caw = np.asarray(inputs["c_attn_w"], np.float32)
    cab = np.asarray(inputs["c_attn_b"], np.float32)
    cpw = np.asarray(inputs["c_proj_w"], np.float32)
    cpb = np.asarray(inputs["c_proj_b"], np.float32)
    mw = np.asarray(inputs["mlp_w"], np.float32)
    mb = np.asarray(inputs["mlp_b"], np.float32)
    ident = np.eye(P, dtype=np.float32)

    in_maps = []
    for c in range(NCORES):
        b, g = c // TP, c % TP
        gs = slice(g * DH, (g + 1) * DH)
        qkb = np.stack([
            cab[g * DH: g * DH + P],
            cab[g * DH + P: (g + 1) * DH],
            cab[H + g * DH: H + g * DH + P],
            cab[H + g * DH + P: H + (g + 1) * DH],
        ], axis=1)  # [P, 4] f32
        in_maps.append({
            "xatt": np.ascontiguousarray(xq[b].T).astype(bf),
            "xatd": np.ascontiguousarray(xk[b].T).astype(bf),
            "wq": np.ascontiguousarray(caw[:, gs]).astype(bf),
            "wk": np.ascontiguousarray(caw[:, H + g * DH: H + (g + 1) * DH]).astype(bf),
            "wv": np.ascontiguousarray(
                caw[:, 2 * H + g * DH: 2 * H + (g + 1) * DH]).astype(bf),
            "qkb": np.ascontiguousarray(qkb),
            "vb": np.ascontiguousarray(
                cab[None, 2 * H + g * DH: 2 * H + (g + 1) * DH]).astype(bf),
            "maskrep": np.ascontiguousarray(
                np.broadcast_to(mask[b][None, :], (P, S))).astype(bf),
            "cprojw": np.ascontiguousarray(cpw[:, gs]).astype(bf),
            "cprojb": np.ascontiguousarray(cpb[gs].reshape(2, P).T),
            "mlpw": np.ascontiguousarray(mw[:, gs]).astype(bf),
            "mlpb": np.ascontiguousarray(mb[gs].reshape(2, P).T),
            "ident": np.ascontiguousarray(ident).astype(bf),
            "cprojwf": np.ascontiguousarray(cpw).astype(bf),
            "cprojbf": np.ascontiguousarray(cpb.reshape(NF, P).T),
        })
    return in_maps


# revision 38
# speedup vs baseline: 1.7741x; 1.7741x over previous
"""Trainium2 Bass kernel for nn_Attention_52536039965434 (v2).

Reference computation (B=2, SQ=SK=2048, H=1024, NH=16, HD=64):
    qkv = x @ c_attn_w + b ; per-head attention with multiplicative mask
    (post-score, pre-softmax); attn @ c_proj_w + b; gelu(cat(x, attn) @ mlp_w + b)

Sharding (8 cores): core c -> (b = c//4, g = c%4). Data parallel over batch,
tensor parallel over 4 head-groups (4 heads = 256 dims each).

v2 design vs v1:
  - all matmul operands bf16 (host converts); psum stays f32. Halves DMA,
    SBUF and collective bytes; required for the 65-wide PV matmuls.
  - PV runs q-major: out [q-tile 128, 4 heads x 65] with probs as lhsT,
    halving PV PE cycles vs the [65, q] layout. Denominator = per-head ones
    column of V. Normalize on DVE with per-partition reciprocal scalars.
  - attn tiles are PE-transposed back to feature-major for c_proj/AllGather.
  - exp on ACT in [128, 1024] tiles (2 heads per tile), bf16 out.
  - biases folded into DVE evacuations (no augmented-row matmuls).
  - gelu deferred to the tail (single ACT table switch away from Exp).
  - AllGathers split h0={qb0,qb1}, q2, q3 so c_proj/mlp2 of early segments
    fill PE gaps during later attention blocks; mlp1/Q interleaved as PE
    filler inside the exp-bound attention phase.
"""

import os

import numpy as np

import concourse.bacc as bacc
import concourse.mybir as mybir
import concourse.tile as tile
from concourse import bass_utils

# ---- problem dims (hardcoded per contest contract) ----
B = 2
S = 2048          # SQ == SK
H = 1024
NH = 16
HD = 64
NCORES = 8
TP = 4            # cores per batch (head groups)
HPC = NH // TP    # heads per core = 4
DH = HPC * HD     # per-core head width = 256
QB = 512          # q-block
P = 128
NKT = S // P      # 16 k-tiles
NQB = S // QB     # 4 q-blocks
NF = H // P       # 8 feature tiles

F32 = mybir.dt.float32
BF16 = mybir.dt.bfloat16
AF = mybir.ActivationFunctionType
ALU = mybir.AluOpType

# tail segments: one per q-block. AG1(i) fires at qb_i's end, so consumers
# sit ~2 q-blocks (50us) behind their collective - no in-order stalls.
SEGS = [(0, 1), (1, 1), (2, 1), (3, 1)]


def _build_nc(reps=1, ag_mode=None):
    if ag_mode is None:
        ag_mode = os.environ.get("KERNEL_AG", "cc")  # cc | dma (timing expt)

    nc = bacc.Bacc(
        "TRN2", target_bir_lowering=False, debug=False, num_devices=NCORES
    )

    # ---- kernel I/O (per-core contents supplied via in_maps) ----
    xatt_d = nc.dram_tensor("xatt", [H, S], BF16, kind="ExternalInput").ap()
    xatd_d = nc.dram_tensor("xatd", [H, S], BF16, kind="ExternalInput").ap()
    wq_d = nc.dram_tensor("wq", [H, DH], BF16, kind="ExternalInput").ap()
    wk_d = nc.dram_tensor("wk", [H, DH], BF16, kind="ExternalInput").ap()
    wv_d = nc.dram_tensor("wv", [H, DH], BF16, kind="ExternalInput").ap()
    qkb_d = nc.dram_tensor("qkb", [P, 4], F32, kind="ExternalInput").ap()
    vb_d = nc.dram_tensor("vb", [1, DH], BF16, kind="ExternalInput").ap()
    mask_d = nc.dram_tensor("maskrep", [P, S], BF16, kind="ExternalInput").ap()
    cpw_d = nc.dram_tensor("cprojw", [H, DH], BF16, kind="ExternalInput").ap()
    cpb_d = nc.dram_tensor("cprojb", [P, 2], F32, kind="ExternalInput").ap()
    mw_d = nc.dram_tensor("mlpw", [2 * H, DH], BF16, kind="ExternalInput").ap()
    mb_d = nc.dram_tensor("mlpb", [P, 2], F32, kind="ExternalInput").ap()
    id_d = nc.dram_tensor("ident", [P, P], BF16, kind="ExternalInput").ap()
    cpwf_d = nc.dram_tensor("cprojwf", [H, H], BF16, kind="ExternalInput").ap()
    cpbf_d = nc.dram_tensor("cprojbf", [P, NF], F32, kind="ExternalInput").ap()
    outT = nc.dram_tensor("outT", [DH, S], F32, kind="ExternalOutput").ap()

    rg = [[0, 1, 2, 3], [4, 5, 6, 7]]

    def allgather(src_ap, dst_ap):
        if ag_mode == "cc":
            nc.gpsimd.collective_compute(
                "AllGather", ALU.bypass, replica_groups=rg,
                ins=[src_ap.opt()], outs=[dst_ap.opt()],
            )
        else:
            # timing experiment: same bytes landed, no collective (wrong vals)
            for r in range(4):
                nc.gpsimd.dma_start(
                    out=dst_ap[r * DH: (r + 1) * DH, :], in_=src_ap[:])

    with tile.TileContext(nc) as tc:
      for rep in range(reps):
        with (
            tc.tile_pool(name=f"w{rep}", bufs=1) as wpool,
            tc.tile_pool(name=f"per{rep}", bufs=1) as per,
            tc.tile_pool(name=f"xch{rep}", bufs=3) as xch_pool,
            tc.tile_pool(name=f"et{rep}", bufs=4) as etp,
            tc.tile_pool(name=f"aq{rep}", bufs=2) as aqp,
            tc.tile_pool(name=f"sm{rep}", bufs=2) as smp,
            tc.tile_pool(name=f"zl{rep}", bufs=1) as zlp,
            tc.tile_pool(name=f"go{rep}", bufs=2) as gop,
            tc.tile_pool(name=f"dram{rep}", bufs=1, space="DRAM") as dram,
            tc.tile_pool(name=f"psSC{rep}", bufs=2, space="PSUM") as psSC,
            tc.tile_pool(name=f"psPV{rep}", bufs=1, space="PSUM") as psPV,
        ):
            # ---------------- weight / persistent loads ----------------
            wq_sb = wpool.tile([P, NF * DH], BF16, tag="wq")
            wk_sb = wpool.tile([P, NF * DH], BF16, tag="wk")
            wv_sb = wpool.tile([P, NF * DH], BF16, tag="wv")
            cproj_sb = wpool.tile([P, NF * DH], BF16, tag="cproj")
            mlp_sb = wpool.tile([P, 2 * NF * DH], BF16, tag="mlp")
            mask_sb = wpool.tile([P, S], BF16, tag="mask")
            ident_sb = wpool.tile([P, P], BF16, tag="ident")
            qkb_sb = wpool.tile([P, 4], F32, tag="qkb")
            cpb_sb = wpool.tile([P, 2], F32, tag="cpb")
            mb_sb = wpool.tile([P, 2], F32, tag="mb")
            vb_sb = wpool.tile([1, DH], BF16, tag="vb")
            vbb_sb = wpool.tile([P, DH], BF16, tag="vbb")
            cpwf_sb = wpool.tile([P, NF * H], BF16, tag="cpwf")
            cpbf_sb = wpool.tile([P, NF], F32, tag="cpbf")
            xatt_sb = per.tile([P, NF * S], BF16, tag="xatt")

            def load_w(w_d, w_sb):
                nc.sync.dma_start(
                    out=w_sb[:].rearrange("p (t d) -> p t d", d=DH),
                    in_=w_d[:].rearrange("(t p) d -> p t d", p=P),
                )

            def load_xatt(qb):
                cs = slice(qb * QB, (qb + 1) * QB)
                nc.sync.dma_start(
                    out=xatt_sb[:].rearrange("p (t q) -> p t q", q=S)
                    [:, :, cs],
                    in_=xatt_d[:].rearrange("(t p) q -> p t q", p=P)[:, :, cs],
                )

            def load_cold():
                # deferred until after the hot path is queued
                for qb in range(1, NQB):
                    load_xatt(qb)
                nc.sync.dma_start(
                    out=mlp_sb[:].rearrange("p (t d) -> p t d", d=DH),
                    in_=mw_d[:].rearrange("(t p) d -> p t d", p=P),
                )
                load_w(cpw_d, cproj_sb)
                nc.sync.dma_start(
                    out=cpwf_sb[:].rearrange("p (t d) -> p t d", d=H),
                    in_=cpwf_d[:].rearrange("(t p) d -> p t d", p=P),
                )
                nc.sync.dma_start(out=cpbf_sb[:], in_=cpbf_d[:])

            # persistent activations
            QT_sb = per.tile([P, 2 * S], BF16, tag="qt")    # p-half at p*S
            KT_sb = per.tile([P, 2 * S], BF16, tag="kt")
            V_sb = per.tile([P, NKT * 260], BF16, tag="v")  # kt: 4 heads x 65
            out1_sb = per.tile([P, 2 * S], BF16, tag="out1")  # mlp acc

            # ones columns of augmented V (denominator trick)
            nc.vector.memset(
                V_sb[:].rearrange("p (k h c) -> p k h c", h=HPC, c=65)
                [:, :, :, 64:65], 1.0)

            # DRAM buffers for collectives, per segment
            ag1_in, ag1_out, ag2_in, ag2_out = [], [], [], []
            for si, (q0, nqs) in enumerate(SEGS):
                L = nqs * QB
                ag1_in.append(dram.tile([DH, L], BF16, tag=f"a1i{si}",
                                        name=f"ag1_in{si}"))
                ag1_out.append(dram.tile([H, L], BF16, tag=f"a1o{si}",
                                         name=f"ag1_out{si}"))
                if si < 2:
                    ag2_in.append(dram.tile([DH, L], BF16, tag=f"a2i{si}",
                                            name=f"ag2_in{si}"))
                    ag2_out.append(dram.tile([H, L], BF16, tag=f"a2o{si}",
                                             name=f"ag2_out{si}"))
            ah_sb = [per.tile([P, NF * nqs * QB], BF16, tag=f"ah{si}",
                              name=f"ah{si}")
                     for si, (q0, nqs) in enumerate(SEGS)]
            z_sb = [per.tile([P, 2 * QB], BF16, tag=f"z{si}", name=f"z{si}")
                    for si in range(2)]

            def wsl(w_sb, t, p):
                return w_sb[:, t * DH + p * P: t * DH + (p + 1) * P]

            acc_n = [0]

            def acc_tile(w=QB):
                # fillers/K/tail GEMM accumulators live in the sc psum ring
                acc_n[0] += 1
                return psSC.tile([P, 2 * QB], F32, tag="sc",
                                 name=f"acc{acc_n[0]}")[:, 0:w]

            def emit_pv(ets, pv_tiles, kt):
                # PV for one kt step: 4 heads x 4 q-subtiles, 65-wide frees.
                # One accumulation group per psum bank: start only zeroes the
                # 2KB zero-region once (kt==0, h==0); later first-writes land
                # on pending-zero bytes, so plain accumulate is correct.
                for p in range(2):
                    et = ets[(kt, p)]
                    for h2 in range(2):
                        h = 2 * p + h2
                        for qt in range(4):
                            nc.tensor.matmul(
                                pv_tiles[qt][:, h * 65: (h + 1) * 65],
                                lhsT=et[:, h2 * QB + qt * P:
                                        h2 * QB + (qt + 1) * P],
                                rhs=V_sb[:, kt * 260 + h * 65:
                                         kt * 260 + (h + 1) * 65],
                                start=(kt == 0 and h == 0),
                                stop=(kt == NKT - 1 and h == 3),
                                skip_group_check=True)

            # ---------------- filler units (emitted lazily) -------------
            HQ = QB // 2   # 256-col filler sub-unit width

            def mlp1_unit(qb, ct, hf):
                o = qb * QB + hf * HQ
                def emit():
                    ps = acc_tile(HQ)
                    for t in range(NF):
                        nc.tensor.matmul(
                            ps[:], lhsT=wsl(mlp_sb, t, ct),
                            rhs=xatt_sb[:, t * S + o: t * S + o + HQ],
                            start=(t == 0), stop=(t == NF - 1))
                    nc.vector.tensor_scalar_add(
                        out1_sb[:, ct * S + o: ct * S + o + HQ],
                        ps[:], mb_sb[:, ct: ct + 1])
                return emit

            def q_unit(qb, p, hf):
                o = qb * QB + hf * HQ
                def emit():
                    ps = acc_tile(HQ)
                    for t in range(NF):
                        nc.tensor.matmul(
                            ps[:], lhsT=wsl(wq_sb, t, p),
                            rhs=xatt_sb[:, t * S + o: t * S + o + HQ],
                            start=(t == 0), stop=(t == NF - 1))
                    nc.vector.tensor_scalar_add(
                        QT_sb[:, p * S + o: p * S + o + HQ],
                        ps[:], qkb_sb[:, p: p + 1])
                return emit

            def cproj_unit(si, qb, ot, hf):
                q0, nqs = SEGS[si]
                qo = (qb - q0) * QB + hf * HQ
                L = nqs * QB
                def emit():
                    ps = acc_tile(HQ)
                    for t in range(NF):
                        nc.tensor.matmul(
                            ps[:], lhsT=wsl(cproj_sb, t, ot),
                            rhs=ah_sb[si][:, t * L + qo: t * L + qo + HQ],
                            start=(t == 0), stop=(t == NF - 1))
                    nc.vector.tensor_scalar_add(
                        z_sb[si][:, ot * L + qo: ot * L + qo + HQ],
                        ps[:], cpb_sb[:, ot: ot + 1])
                return emit

            def mlp2_unit(si, qb, ct, hf, zch):
                o = hf * HQ
                def emit():
                    z = zchs[qb] if zch is None else zch
                    ps = acc_tile(HQ)
                    for t in range(NF):
                        nc.tensor.matmul(
                            ps[:], lhsT=wsl(mlp_sb, NF + t, ct),
                            rhs=z[:, t * QB + o: t * QB + o + HQ],
                            start=(t == 0), stop=(t == NF - 1))
                    sl = out1_sb[:, ct * S + qb * QB + o:
                                 ct * S + qb * QB + o + HQ]
                    nc.vector.tensor_tensor(sl, ps[:], sl, ALU.add)
                return emit

            # ------------- helpers for the interleaved main phase -------
            def load_xch(kb):
                x_ch = xch_pool.tile([P, NF * QB], BF16, tag="xch",
                                     name=f"xd{kb}")
                cs = slice(kb * QB, (kb + 1) * QB)
                nc.sync.dma_start(
                    out=x_ch[:].rearrange("p (t q) -> p t q", q=QB),
                    in_=xatd_d[:].rearrange("(t p) q -> p t q", p=P)[:, :, cs],
                )
                return x_ch

            def k_group(x_ch, kb, p):
                ps = acc_tile()
                for t in range(NF):
                    nc.tensor.matmul(
                        ps[:], lhsT=wsl(wk_sb, t, p),
                        rhs=x_ch[:, t * QB: (t + 1) * QB],
                        start=(t == 0), stop=(t == NF - 1))
                # (ps + k_bias) * mask, bf16 out
                nc.vector.scalar_tensor_tensor(
                    KT_sb[:, p * S + kb * QB: p * S + (kb + 1) * QB],
                    ps[:], qkb_sb[:, 2 + p: 3 + p],
                    mask_sb[:, kb * QB: (kb + 1) * QB],
                    ALU.add, ALU.mult)

            def v_group(x_ch, kb, sub):
                kt = kb * (QB // P) + sub
                acc_n[0] += 1
                psv = psSC.tile([P, 2 * QB], F32, tag="sc",
                                name=f"psv{acc_n[0]}")[:, 0:DH]
                for t in range(NF):
                    nc.tensor.matmul(
                        psv[:],
                        lhsT=x_ch[:, t * QB + sub * P: t * QB + (sub + 1) * P],
                        rhs=wv_sb[:, t * DH: (t + 1) * DH],
                        start=(t == 0), stop=(t == NF - 1))
                nc.vector.tensor_tensor(
                    V_sb[:, kt * 260: (kt + 1) * 260]
                    .rearrange("p (h c) -> p h c", c=65)[:, :, 0:64],
                    psv.rearrange("p (h c) -> p h c", c=HD),
                    vbb_sb[:].rearrange("p (h c) -> p h c", c=HD),
                    ALU.add)

            def sc_step(qb, kt, ets):
                for p in range(2):
                    sc = psSC.tile([P, 2 * QB], F32, tag="sc",
                                   name=f"sc{qb}_{kt}_{p}")
                    for h2 in range(2):
                        nc.tensor.matmul(
                            sc[:, h2 * QB: (h2 + 1) * QB],
                            lhsT=KT_sb[64 * h2: 64 * h2 + 64,
                                       p * S + kt * P: p * S + (kt + 1) * P],
                            rhs=QT_sb[64 * h2: 64 * h2 + 64,
                                      p * S + qb * QB: p * S + (qb + 1) * QB],
                            start=True, stop=True,
                            tile_position=(64 * h2, 0))
                    et = etp.tile([P, 2 * QB], BF16, tag="et",
                                  name=f"et{qb}_{kt}_{p}")
                    nc.scalar.activation(et[:], sc[:], AF.Exp)
                    ets[(kt, p)] = et

            def seg_of(qb):
                for si, (q0, nqs) in enumerate(SEGS):
                    if q0 <= qb < q0 + nqs:
                        return si
                raise AssertionError

            def emit_ag1(si):
                allgather(ag1_in[si][:], ag1_out[si][:])
                q0, nqs = SEGS[si]
                L = nqs * QB
                nc.sync.dma_start(
                    out=ah_sb[si][:].rearrange("p (t q) -> p t q", q=L),
                    in_=ag1_out[si][:].rearrange("(t p) q -> p t q", p=P),
                )

            def emit_ag2(si):
                q0, nqs = SEGS[si]
                L = nqs * QB
                nc.sync.dma_start(
                    out=ag2_in[si][:].rearrange("(o p) q -> p o q", p=P),
                    in_=z_sb[si][:].rearrange("p (o q) -> p o q", q=L),
                )
                allgather(ag2_in[si][:], ag2_out[si][:])

            def normalize_ship(qb, si, q0, pv_tiles):
                for qt in range(4):
                    pv = pv_tiles[qt][:]
                    rec = smp.tile([P, 4], F32, tag="rec")
                    nc.vector.reciprocal(
                        rec[:].rearrange("p (h c) -> p h c", c=1),
                        pv.rearrange("p (h c) -> p h c", c=65)[:, :, 64:65])
                    aq = aqp.tile([P, DH], BF16, tag="attq")
                    for h in range(HPC):
                        nc.vector.tensor_scalar_mul(
                            aq[:, h * HD: (h + 1) * HD],
                            pv[:, h * 65: h * 65 + 64],
                            rec[:, h: h + 1])
                    # XBAR DMA transpose to feature-major, then ship to DRAM
                    at = aqp.tile([P, DH], BF16, tag="attT")
                    for f in range(2):
                        nc.sync.dma_start_transpose(
                            at[:, f * P: (f + 1) * P],
                            aq[:, f * P: (f + 1) * P])
                    qcol = (qb - q0) * QB + qt * P
                    nc.sync.dma_start(
                        out=ag1_in[si][:].rearrange("(t p) q -> p t q", p=P)
                        [:, :, qcol: qcol + P],
                        in_=at[:].rearrange("p (t q) -> p t q", q=P))

            # -------- main phase: K/V production interleaved with qb0 ------
            # hot loads first: K path, then Q path, then V
            load_w(wk_d, wk_sb)
            x_chs = {0: load_xch(0)}
            nc.sync.dma_start(out=qkb_sb[:], in_=qkb_d[:])
            nc.sync.dma_start(out=mask_sb[:], in_=mask_d[:])
            load_w(wq_d, wq_sb)
            load_xatt(0)
            x_chs[1] = load_xch(1)
            load_w(wv_d, wv_sb)
            nc.sync.dma_start(out=vb_sb[:], in_=vb_d[:])
            nc.gpsimd.partition_broadcast(vbb_sb[:], vb_sb[:], channels=P)
            nc.sync.dma_start(out=ident_sb[:], in_=id_d[:])
            nc.sync.dma_start(out=cpb_sb[:], in_=cpb_d[:])
            nc.sync.dma_start(out=mb_sb[:], in_=mb_d[:])
            k_group(x_chs[0], 0, 0)
            k_group(x_chs[0], 0, 1)
            for hf in range(2):
                q_unit(0, 0, hf)()
                q_unit(0, 1, hf)()

            pv_tiles = [
                psPV.tile([P, HPC * 65], F32, tag=f"pv{qt}", name=f"pv0_{qt}")
                for qt in range(4)]
            ets = {}
            for kb in range(NQB):
                if kb + 2 < NQB:
                    x_chs[kb + 2] = load_xch(kb + 2)
                for j in range(QB // P):
                    kt = kb * (QB // P) + j
                    sc_step(0, kt, ets)
                    if kt > 0:
                        emit_pv(ets, pv_tiles, kt - 1)
                    v_group(x_chs[kb], kb, j)
                    if kb + 1 < NQB and j in (1, 2):
                        k_group(x_chs[kb + 1], kb + 1, j - 1)
                if kb == 0:
                    load_cold()
            emit_pv(ets, pv_tiles, NKT - 1)
            for hf in range(2):
                q_unit(1, 0, hf)()
                q_unit(1, 1, hf)()
            normalize_ship(0, 0, 0, pv_tiles)
            emit_ag1(0)

            # ---------------- qb 1..3: attention + fillers ---------------
            # filler placement: {qb: {after-kt: [unit, ...]}}
            fill = {qb: {} for qb in range(NQB)}

            def sched(qb, units, kt0=1):
                for i, u in enumerate(units):
                    fill[qb].setdefault(kt0 + i, []).append(u)

            zchs = {}

            def zch_load(si, qb):
                def emit():
                    zch = zlp.tile([P, NF * QB], BF16, tag="zch",
                                   name=f"zch{qb}")
                    nc.sync.dma_start(
                        out=zch[:].rearrange("p (t q) -> p t q", q=QB),
                        in_=ag2_out[si][:].rearrange("(t p) q -> p t q", p=P))
                    zchs[qb] = zch
                return emit

            def zfull_group(si, ot8, dst):
                # full c_proj for one 128-row output tile of a 512-q segment
                def emit():
                    ps = acc_tile()
                    for t in range(NF):
                        nc.tensor.matmul(
                            ps[:],
                            lhsT=cpwf_sb[:, t * H + ot8 * P:
                                         t * H + (ot8 + 1) * P],
                            rhs=ah_sb[si][:, t * QB: (t + 1) * QB],
                            start=(t == 0), stop=(t == NF - 1))
                    nc.vector.tensor_scalar_add(
                        dst[:, ot8 * QB: (ot8 + 1) * QB],
                        ps[:], cpbf_sb[:, ot8: ot8 + 1])
                return emit

            z23f = zlp.tile([P, NF * QB], BF16, tag="z23f")

            sched(1, [mlp1_unit(0, 0, 0), mlp1_unit(0, 0, 1),
                      mlp1_unit(0, 1, 0), mlp1_unit(0, 1, 1),
                      q_unit(2, 0, 0), q_unit(2, 0, 1),
                      q_unit(2, 1, 0), q_unit(2, 1, 1),
                      mlp1_unit(1, 0, 0), mlp1_unit(1, 0, 1),
                      mlp1_unit(1, 1, 0), mlp1_unit(1, 1, 1),
                      mlp1_unit(2, 0, 0), mlp1_unit(2, 0, 1)])
            sched(2, [q_unit(3, 0, 0), q_unit(3, 0, 1),
                      q_unit(3, 1, 0), q_unit(3, 1, 1),
                      cproj_unit(0, 0, 0, 0), cproj_unit(0, 0, 0, 1),
                      cproj_unit(0, 0, 1, 0), cproj_unit(0, 0, 1, 1)])
            fill[2].setdefault(8, []).append(lambda: emit_ag2(0))
            sched(2, [mlp1_unit(2, 1, 0), mlp1_unit(2, 1, 1),
                      mlp1_unit(3, 0, 0), mlp1_unit(3, 0, 1),
                      mlp1_unit(3, 1, 0), mlp1_unit(3, 1, 1)], kt0=9)
            fill[3].setdefault(1, []).append(zch_load(0, 0))
            sched(3, [cproj_unit(1, 1, 0, 0), cproj_unit(1, 1, 0, 1),
                      cproj_unit(1, 1, 1, 0), cproj_unit(1, 1, 1, 1)])
            fill[3].setdefault(4, []).append(lambda: emit_ag2(1))
            sched(3, [mlp2_unit(0, 0, 0, 0, None), mlp2_unit(0, 0, 0, 1, None),
                      mlp2_unit(0, 0, 1, 0, None), mlp2_unit(0, 0, 1, 1, None)],
                  kt0=5)
            # q2 z via local full c_proj (ah2 ready early in qb3)
            sched(3, [zfull_group(2, i, z23f) for i in range(6)], kt0=9)

            for qb in range(1, NQB):
                si = seg_of(qb)
                q0, nqs = SEGS[si]
                pv_tiles = [
                    psPV.tile([P, HPC * 65], F32, tag=f"pv{qt}",
                              name=f"pv{qb}_{qt}")
                    for qt in range(4)]
                ets = {}
                for kt in range(NKT):
                    sc_step(qb, kt, ets)
                    if kt > 0:
                        emit_pv(ets, pv_tiles, kt - 1)
                        for e in fill[qb].get(kt - 1, []):
                            e()
                emit_pv(ets, pv_tiles, NKT - 1)
                for e in fill[qb].get(NKT - 1, []):
                    e()
                normalize_ship(qb, si, q0, pv_tiles)
                if qb == q0 + nqs - 1:
                    emit_ag1(si)

            # ---------------- tail ----------------
            for i in range(6, NF):
                zfull_group(2, i, z23f)()
            zch_load(1, 1)()
            for ct in range(2):
                for hf in range(2):
                    mlp2_unit(1, 1, ct, hf, None)()    # qb1
            for ct in range(2):
                for hf in range(2):
                    mlp2_unit(2, 2, ct, hf, z23f)()    # q2
            gelu_exact = os.environ.get("KERNEL_GELU", "builtin") == "exact"

            def gelu_out(qb, ct):
                go = gop.tile([P, QB], F32, tag="gout")
                gin = out1_sb[:, ct * S + qb * QB: ct * S + (qb + 1) * QB]
                if not gelu_exact:
                    nc.scalar.activation(go[:], gin, AF.Gelu_apprx_tanh)
                else:
                    # exact GPT-2 tanh gelu from primitives (CoreSim path)
                    u = gop.tile([P, QB], F32, tag="gu")
                    nc.vector.tensor_mul(u[:], gin, gin)             # x^2
                    nc.vector.tensor_mul(u[:], u[:], gin)            # x^3
                    nc.vector.scalar_tensor_tensor(
                        u[:], u[:], 0.044715, gin, ALU.mult, ALU.add)
                    nc.scalar.activation(
                        go[:], u[:], AF.Tanh, scale=0.7978845608028654)
                    nc.vector.scalar_tensor_tensor(
                        go[:], go[:], 1.0, gin, ALU.add, ALU.mult)   # (1+t)*x
                    nc.vector.tensor_scalar_mul(go[:], go[:], 0.5)
                nc.sync.dma_start(
                    out=outT[ct * P: (ct + 1) * P, qb * QB: (qb + 1) * QB],
                    in_=go[:])

            for qb in range(3):
                for ct in range(2):
                    gelu_out(qb, ct)
            # q3 z via local full c_proj (reuses z23f after mlp2_q2 reads)
            for i in range(NF):
                zfull_group(3, i, z23f)()
            for ct in range(2):
                for hf in range(2):
                    mlp2_unit(3, 3, ct, hf, z23f)()
            for ct in range(2):
                gelu_out(3, ct)

    nc.compile()
    return nc


_NC_CACHE = {}
LAST_RESULTS = None


def _get_nc():
    if 1 not in _NC_CACHE:
        _NC_CACHE[1] = _build_nc()
    return _NC_CACHE[1]


def _get_nc_reps(reps):
    key = ("reps", reps)
    if key not in _NC_CACHE:
        _NC_CACHE[key] = _build_nc(reps=reps)
    return _NC_CACHE[key]


def kernel(**inputs):
    global LAST_RESULTS
    nc = _get_nc()
    in_maps = make_in_maps(inputs)

    trace = bool(int(os.environ.get("KERNEL_TRACE", "0")))
    res = bass_utils.run_bass_kernel_spmd(
        nc, in_maps, core_ids=list(range(NCORES)), trace=trace
    )
    LAST_RESULTS = res

    out = np.empty((B, S, H), np.float32)
    for c in range(NCORES):
        b, g = c // TP, c % TP
        out[b, :, g * DH: (g + 1) * DH] = res.results[c]["outT"].T
    return out


def make_in_maps(inputs):
    bf = mybir.dt.np(BF16)
    xq = np.asarray(inputs["attender_seq"], np.float32)
    xk = np.asarray(inputs["attendee_seq"], np.float32)
    mask = np.asarray(inputs["attendee_mask"]).astype(np.float32)
    caw = np.asarray(inputs["c_attn_w"], np.float32)
    cab = np.asarray(inputs["c_attn_b"], np.float32)
    cpw = np.asarray(inputs["c_proj_w"], np.float32)
    cpb = np.asarray(inputs["c_proj_b"], np.float32)
    mw = np.asarray(inputs["mlp_w"], np.float32)
    mb = np.asarray(inputs["mlp_b"], np.float32)
    ident = np.eye(P, dtype=np.float32)

    in_maps = []
    for c in range(NCORES):
        b, g = c // TP, c % TP
        gs = slice(g * DH, (g + 1) * DH)
        qkb = np.stack([
            cab[g * DH: g * DH + P],
            cab[g * DH + P: (g + 1) * DH],
            cab[H + g * DH: H + g * DH + P],
            cab[H + g * DH + P: H + (g + 1) * DH],
        ], axis=1)  # [P, 4] f32
        in_maps.append({
            "xatt": np.ascontiguousarray(xq[b].T).astype(bf),
            "xatd": np.ascontiguousarray(xk[b].T).astype(bf),
            "wq": np.ascontiguousarray(caw[:, gs]).astype(bf),
            "wk": np.ascontiguousarray(caw[:, H + g * DH: H + (g + 1) * DH]).astype(bf),
            "wv": np.ascontiguousarray(
                caw[:, 2 * H + g * DH: 2 * H + (g + 1) * DH]).astype(bf),
            "qkb": np.ascontiguousarray(qkb),
            "vb": np.ascontiguousarray(
                cab[None, 2 * H + g * DH: 2 * H + (g + 1) * DH]).astype(bf),
            "maskrep": np.ascontiguousarray(
                np.broadcast_to(mask[b][None, :], (P, S))).astype(bf),
            "cprojw": np.ascontiguousarray(cpw[:, gs]).astype(bf),
            "cprojb": np.ascontiguousarray(cpb[gs].reshape(2, P).T),
            "mlpw": np.ascontiguousarray(mw[:, gs]).astype(bf),
            "mlpb": np.ascontiguousarray(mb[gs].reshape(2, P).T),
            "ident": np.ascontiguousarray(ident).astype(bf),
            "cprojwf": np.ascontiguousarray(cpw).astype(bf),
            "cprojbf": np.ascontiguousarray(cpb.reshape(NF, P).T),
        })
    return in_maps
